# revision 9
# baseline (speedup 1.0000x reference)
"""Trainium2 Bass kernel for Conv2dWeightModulate (no style).

The reference computes an equalized-lr + demodulated 3x3 conv:
    w = weight * C_EQ;  w *= rsqrt(sum(w^2, (I,K,K)) + eps);  out = conv2d(x, w, pad=1)

The tiny weight normalization runs on host (numpy); the conv runs on 8
NeuronCores, data-parallel over the batch (2 images per core).

Host-side data layout: x is cast to bf16 and split by row parity into
xP[b, c, p, h2, w] (= x[b, c, 2*h2+p, w]) so every DMA reads long
contiguous spans; the device writes a parity-split fp16 output that the
host re-interleaves (and upcasts to fp32).

Device kernel layout (per core):
  x is stored in SBUF parity-interleaved: partitions 0-63 hold the 64
  channels of even image rows, partitions 64-127 the odd rows, with each
  row padded to 258 columns (zero borders give the conv its padding).
  Chunk column s of a block with row base R holds:
      half A (parts 0:64):   x row R + 2(s-1)
      half B (parts 64:128): x row R + 2s - 1
  so chunk s aligns x rows (2j, 2j+1) vertically.  A 3x3 conv then becomes,
  per pair of same-parity output rows (one 512-wide matmul free dim):
      - even rows: K=128 matmul (taps kh=1+kh=2) x3 kw  +  K=64 (kh=0) x3
      - odd rows:  K=128 matmul (taps kh=0+kh=1) x3 kw  +  K=64 (kh=2) x3
  Adjacent row-pairs are col-tiled (tile_position via PSUM base partition
  64) so the pair runs concurrently on disjoint PE column groups; the
  K=64 leftovers of even/odd chunks land on disjoint PE quadrants and run
  4-way concurrent.  Each even/odd PSUM pair shares one full-width
  [128, 2, W] PSUM tile (halves on disjoint partition ranges), so copies
  are full-128-lane and only 2 PSUM banks are live per row-group,
  letting the pool double-buffer 4 deep.  Accumulation is fp32 in PSUM;
  outputs staged through SBUF as fp16 in 32-row groups and DMAed out.
"""

import numpy as np

IN_F = 64
OUT_F = 64
KS = 3
EPS = 1e-05
C_EQ = 1.0 / np.sqrt(IN_F * KS * KS)

B_FULL = 16
H_FULL = 256
W = 256
N_CORES = 8
CW = W + 2  # padded row width


def build_nc(bpc, h, block=64):
    """Build the per-core Bass program: bpc images of [64, h, 256] each."""
    from concourse import bacc
    import concourse.mybir as mybir
    from concourse.tile import TileContext

    assert h % block == 0 and block % 32 == 0
    nblk = h // block
    ngrp = block // 32  # 32-row output staging groups per block
    sch = block // 2 + 2  # chunk columns per x tile
    f32 = mybir.dt.float32
    bf16 = mybir.dt.bfloat16
    f16 = mybir.dt.float16

    nc = bacc.Bacc("TRN2", target_bir_lowering=False, debug=False)
    x = nc.dram_tensor("x", [bpc, IN_F, 2, h // 2, CW], bf16, kind="ExternalInput")
    wp = nc.dram_tensor("wpack", [128, 9, 64], bf16, kind="ExternalInput")
    # output stays in staging order: h = 2*(16*hg + 4*uu + 2*ud + up) + p,
    # so each 32-row group+parity is one fully contiguous 512KB DMA
    # (partition-major, 4KB contiguous per partition); host reassembles.
    # A staging tile's partition half selects ud, its inner row dim is up.
    out = nc.dram_tensor(
        "out", [bpc, 2, h // 32, 2, OUT_F, 4, 2, W], f16, kind="ExternalOutput"
    )
    outr = out.ap().rearrange("b p hg ud c uu up w -> b p hg (ud c) uu up w")

    with TileContext(nc) as tc:
        with (
            tc.tile_pool(name="xp", bufs=5) as xpool,
            tc.tile_pool(name="wpool", bufs=1) as wpool,
            tc.tile_pool(name="st", bufs=8) as spool,
            tc.tile_pool(name="ps", bufs=4, space="PSUM") as ppool,
        ):
            wt = wpool.tile([128, 9, 64], bf16)
            # weights lead the scalar HWDGE queue; the sync HWDGE queue
            # starts on the first x tile in parallel
            nc.scalar.dma_start(out=wt[:], in_=wp.ap())
            for b in range(bpc):
                for blk in range(nblk):
                    R = blk * block
                    h0 = R // 2
                    xt = xpool.tile([128, sch, CW], bf16, tag="xt")
                    # host pre-pads rows to 258 with zero borders, so every
                    # transfer is one contiguous span per channel
                    # half A <- even x rows R .. R+block (chunks 1..sch-1)
                    # half B <- odd x rows R-1 .. R+block-1 (chunks 0..sch-2)
                    if blk == nblk - 1:
                        a_lo, a_hi = h0, h0 + sch - 2
                        nc.gpsimd.memset(xt[0:64, sch - 1, :], 0.0)
                    else:
                        a_lo, a_hi = h0, h0 + sch - 1
                    if blk == 0:
                        nc.gpsimd.memset(xt[64:128, 0, :], 0.0)
                        b_s, b_lo, b_hi = 1, 0, sch - 2
                    else:
                        b_s, b_lo, b_hi = 0, h0 - 1, h0 + sch - 2
                    if b == 0 and blk == 0:
                        # tiny leading pieces so the first row-group's matmuls
                        # gate on ~400KB instead of the whole 1.1MB tile
                        cuts = [6, 14]
                    else:
                        cuts = [sch // 2]
                    a_cut = [a_lo] + [min(a_lo + c, a_hi) for c in cuts] + [a_hi]
                    b_cut = [b_lo] + [min(b_lo + c, b_hi) for c in cuts] + [b_hi]
                    for lo, hi in zip(a_cut, a_cut[1:]):
                        if hi > lo:
                            nc.sync.dma_start(
                                out=xt[0:64, 1 + (lo - a_lo) : 1 + (hi - a_lo), :],
                                in_=x.ap()[b, :, 0, lo:hi, :],
                            )
                    for lo, hi in zip(b_cut, b_cut[1:]):
                        if hi > lo:
                            # B half rides the scalar HWDGE queue: halves the
                            # per-block input latency vs one queue
                            nc.scalar.dma_start(
                                out=xt[64:128, b_s + (lo - b_lo) : b_s + (hi - b_lo), :],
                                in_=x.ap()[b, :, 1, lo:hi, :],
                            )
                    for g in range(ngrp):
                        hg = (h0 + 16 * g) // 16
                        stE = spool.tile([128, 4, 2, W], f16, tag="stE")
                        stO = spool.tile([128, 4, 2, W], f16, tag="stO")
                        for uu in range(4):
                            r0 = R + 32 * g + 8 * uu
                            s0 = (r0 - R) // 2 + 1  # A-chunk of x row r0
                            psE = ppool.tile([128, 2, W], f32, tag="psE")
                            psO = ppool.tile([128, 2, W], f32, tag="psO")
                            # E mains: out rows (r0, r0+2 | r0+4, r0+6), taps kh=1,2
                            for kw in range(3):
                                st_ = kw == 0
                                nc.tensor.matmul(
                                    psE[0:64], wt[:, kw, :],
                                    xt[:, s0 : s0 + 2, kw : kw + W],
                                    start=st_, stop=False,
                                )
                                nc.tensor.matmul(
                                    psE[64:128], wt[:, kw, :],
                                    xt[:, s0 + 2 : s0 + 4, kw : kw + W],
                                    start=st_, stop=False,
                                )
                            # O mains: out rows (r0+1, r0+3 | r0+5, r0+7), taps kh=0,1
                            for kw in range(3):
                                st_ = kw == 0
                                nc.tensor.matmul(
                                    psO[0:64], wt[:, 3 + kw, :],
                                    xt[:, s0 : s0 + 2, kw : kw + W],
                                    start=st_, stop=False,
                                )
                                nc.tensor.matmul(
                                    psO[64:128], wt[:, 3 + kw, :],
                                    xt[:, s0 + 2 : s0 + 4, kw : kw + W],
                                    start=st_, stop=False,
                                )
                            # leftovers (4-way concurrent PE quadrants):
                            # E: tap kh=0 from half B; O: tap kh=2 from half A
                            for kw in range(3):
                                sp_ = kw == 2
                                nc.tensor.matmul(
                                    psE[0:64], wt[64:128, 6 + kw, :],
                                    xt[64:128, s0 - 1 : s0 + 1, kw : kw + W],
                                    start=False, stop=sp_,
                                )
                                nc.tensor.matmul(
                                    psE[64:128], wt[64:128, 6 + kw, :],
                                    xt[64:128, s0 + 1 : s0 + 3, kw : kw + W],
                                    start=False, stop=sp_,
                                )
                                nc.tensor.matmul(
                                    psO[0:64], wt[0:64, 6 + kw, :],
                                    xt[0:64, s0 + 1 : s0 + 3, kw : kw + W],
                                    start=False, stop=sp_,
                                )
                                nc.tensor.matmul(
                                    psO[64:128], wt[0:64, 6 + kw, :],
                                    xt[0:64, s0 + 3 : s0 + 5, kw : kw + W],
                                    start=False, stop=sp_,
                                )
                            nc.scalar.copy(stE[:, uu], psE[:])
                            nc.vector.tensor_copy(out=stO[:, uu], in_=psO[:])
                        # group output DMAs split across the sync HWDGE queue
                        # (fast, shared with input A) and the gpsimd SWDGE
                        # queue, so the tail drains two queues wide
                        nc.sync.dma_start(out=outr[b, 0, hg], in_=stE[:])
                        nc.gpsimd.dma_start(out=outr[b, 1, hg], in_=stO[:])
    nc.compile()
    return nc


def normalize_weight(weight):
    """Host-side equalized-lr + demodulation of the [O,I,3,3] weight."""
    w = np.asarray(weight, dtype=np.float32) * np.float32(C_EQ)
    sigma_inv = 1.0 / np.sqrt(
        np.sum((w * w).astype(np.float32), axis=(1, 2, 3), keepdims=True) + EPS
    )
    return (w * sigma_inv.astype(np.float32)).astype(np.float32)


def pack_weights(w_norm):
    """Pack normalized [O,I,kh,kw] weights into the [128, 9, 64] SBUF image.

    Column group g = kw for the even-row mains (rows 0:64 <- kh=1,
    rows 64:128 <- kh=2), g = 3+kw for odd-row mains (kh=0 / kh=1),
    g = 6+kw for the leftovers (rows 0:64 <- kh=2, rows 64:128 <- kh=0).
    Each [64, 64] slice is wT = w[:, :, kh, kw].T (contraction dim first).
    """
    wt = np.transpose(w_norm, (2, 3, 1, 0))  # [kh, kw, in, out]
    wpack = np.zeros((128, 9, 64), dtype=np.float32)
    for kw in range(3):
        wpack[0:64, kw] = wt[1, kw]
        wpack[64:128, kw] = wt[2, kw]
        wpack[0:64, 3 + kw] = wt[0, kw]
        wpack[64:128, 3 + kw] = wt[1, kw]
        wpack[0:64, 6 + kw] = wt[2, kw]
        wpack[64:128, 6 + kw] = wt[0, kw]
    return wpack


_NC_CACHE = {}


def _get_nc(bpc, h, block=64):
    key = (bpc, h, block)
    if key not in _NC_CACHE:
        _NC_CACHE[key] = build_nc(bpc, h, block)
    return _NC_CACHE[key]


def split_parity(x_f32):
    """[b, c, h, w] f32 -> bf16 [b, c, 2, h//2, w+2]: row parity split plus
    zero border columns (p=0 even rows, p=1 odd rows)."""
    import ml_dtypes

    b, c, h, w = x_f32.shape
    xb = x_f32.astype(ml_dtypes.bfloat16)
    xP = np.zeros((b, c, 2, h // 2, w + 2), dtype=ml_dtypes.bfloat16)
    xP[:, :, 0, :, 1:-1] = xb[:, :, 0::2]
    xP[:, :, 1, :, 1:-1] = xb[:, :, 1::2]
    return xP


def merge_parity(outP):
    """Device [b, p, hg, ud, c, uu, up, w] -> fp32 [b, c, h, w] with
    h = 2*(16*hg + 4*uu + 2*ud + up) + p."""
    b, p, hg, ud, c, uu, up, w = outP.shape
    o = np.transpose(outP, (0, 4, 2, 5, 3, 6, 1, 7))  # b c hg uu ud up p w
    return np.ascontiguousarray(
        o.reshape(b, c, hg * uu * ud * up * p, w), dtype=np.float32
    )


def kernel(x, weight):
    import ml_dtypes
    from concourse import bass_utils

    x = np.asarray(x, dtype=np.float32)
    weight = np.asarray(weight, dtype=np.float32)
    assert x.shape == (B_FULL, IN_F, H_FULL, W), x.shape

    xP = split_parity(x)
    wpack = pack_weights(normalize_weight(weight)).astype(ml_dtypes.bfloat16)
    bpc = B_FULL // N_CORES
    nc = _get_nc(bpc, H_FULL)
    in_maps = [
        {"x": xP[i * bpc : (i + 1) * bpc], "wpack": wpack} for i in range(N_CORES)
    ]
    res = bass_utils.run_bass_kernel_spmd(nc, in_maps, core_ids=list(range(N_CORES)))
    return np.concatenate([merge_parity(r["out"]) for r in res.results], axis=0)


# revision 11
# speedup vs baseline: 1.0028x; 1.0028x over previous
"""Trainium2 Bass kernel for Conv2dWeightModulate (no style).

The reference computes an equalized-lr + demodulated 3x3 conv:
    w = weight * C_EQ;  w *= rsqrt(sum(w^2, (I,K,K)) + eps);  out = conv2d(x, w, pad=1)

The tiny weight normalization runs on host (numpy); the conv runs on 8
NeuronCores, data-parallel over the batch (2 images per core).

Host-side data layout: x is cast to bf16 and split by row parity into
xP[b, c, p, h2, w] (= x[b, c, 2*h2+p, w]) so every DMA reads long
contiguous spans; the device writes a parity-split fp16 output that the
host re-interleaves (and upcasts to fp32).

Device kernel layout (per core):
  x is stored in SBUF parity-interleaved: partitions 0-63 hold the 64
  channels of even image rows, partitions 64-127 the odd rows, with each
  row padded to 258 columns (zero borders give the conv its padding).
  Chunk column s of a block with row base R holds:
      half A (parts 0:64):   x row R + 2(s-1)
      half B (parts 64:128): x row R + 2s - 1
  so chunk s aligns x rows (2j, 2j+1) vertically.  A 3x3 conv then becomes,
  per pair of same-parity output rows (one 512-wide matmul free dim):
      - even rows: K=128 matmul (taps kh=1+kh=2) x3 kw  +  K=64 (kh=0) x3
      - odd rows:  K=128 matmul (taps kh=0+kh=1) x3 kw  +  K=64 (kh=2) x3
  Adjacent row-pairs are col-tiled (tile_position via PSUM base partition
  64) so the pair runs concurrently on disjoint PE column groups; the
  K=64 leftovers of even/odd chunks land on disjoint PE quadrants and run
  4-way concurrent.  Each even/odd PSUM pair shares one full-width
  [128, 2, W] PSUM tile (halves on disjoint partition ranges), so copies
  are full-128-lane and only 2 PSUM banks are live per row-group,
  letting the pool double-buffer 4 deep.  Accumulation is fp32 in PSUM;
  outputs staged through SBUF as fp16 in 32-row groups and DMAed out.
"""

import numpy as np

IN_F = 64
OUT_F = 64
KS = 3
EPS = 1e-05
C_EQ = 1.0 / np.sqrt(IN_F * KS * KS)

B_FULL = 16
H_FULL = 256
W = 256
N_CORES = 8
CW = W + 2  # padded row width


def build_nc(bpc, h, block=64):
    """Build the per-core Bass program: bpc images of [64, h, 256] each."""
    from concourse import bacc
    import concourse.mybir as mybir
    from concourse.tile import TileContext

    assert h % block == 0 and block % 32 == 0
    nblk = h // block
    ngrp = block // 32  # 32-row output staging groups per block
    sch = block // 2 + 2  # chunk columns per x tile
    f32 = mybir.dt.float32
    bf16 = mybir.dt.bfloat16
    f16 = mybir.dt.float16

    nc = bacc.Bacc("TRN2", target_bir_lowering=False, debug=False)
    x = nc.dram_tensor("x", [bpc, IN_F, 2, h // 2, CW], bf16, kind="ExternalInput")
    wp = nc.dram_tensor("wpack", [128, 9, 64], bf16, kind="ExternalInput")
    # output stays in staging order: h = 2*(16*hg + 4*uu + 2*ud + up) + p,
    # so each 32-row group+parity is one fully contiguous 512KB DMA
    # (partition-major, 4KB contiguous per partition); host reassembles.
    # A staging tile's partition half selects ud, its inner row dim is up.
    out = nc.dram_tensor(
        "out", [bpc, 2, h // 32, 2, OUT_F, 4, 2, W], f16, kind="ExternalOutput"
    )
    outr = out.ap().rearrange("b p hg ud c uu up w -> b p hg (ud c) uu up w")

    with TileContext(nc) as tc:
        with (
            tc.tile_pool(name="xp", bufs=5) as xpool,
            tc.tile_pool(name="wpool", bufs=1) as wpool,
            tc.tile_pool(name="st", bufs=8) as spool,
            tc.tile_pool(name="ps", bufs=4, space="PSUM") as ppool,
        ):
            wt = wpool.tile([128, 9, 64], bf16)
            # weights lead the scalar HWDGE queue; the sync HWDGE queue
            # starts on the first x tile in parallel
            nc.scalar.dma_start(out=wt[:], in_=wp.ap())
            for b in range(bpc):
                for blk in range(nblk):
                    R = blk * block
                    h0 = R // 2
                    xt = xpool.tile([128, sch, CW], bf16, tag="xt")
                    # host pre-pads rows to 258 with zero borders, so every
                    # transfer is one contiguous span per channel
                    # half A <- even x rows R .. R+block (chunks 1..sch-1)
                    # half B <- odd x rows R-1 .. R+block-1 (chunks 0..sch-2)
                    if blk == nblk - 1:
                        a_lo, a_hi = h0, h0 + sch - 2
                        nc.gpsimd.memset(xt[0:64, sch - 1, :], 0.0)
                    else:
                        a_lo, a_hi = h0, h0 + sch - 1
                    if blk == 0:
                        nc.gpsimd.memset(xt[64:128, 0, :], 0.0)
                        b_s, b_lo, b_hi = 1, 0, sch - 2
                    else:
                        b_s, b_lo, b_hi = 0, h0 - 1, h0 + sch - 2
                    if b == 0 and blk == 0:
                        # tiny leading pieces so the first row-group's matmuls
                        # gate on ~400KB instead of the whole 1.1MB tile
                        cuts = [6, 14]
                    else:
                        cuts = [sch // 2]
                    a_cut = [a_lo] + [min(a_lo + c, a_hi) for c in cuts] + [a_hi]
                    b_cut = [b_lo] + [min(b_lo + c, b_hi) for c in cuts] + [b_hi]
                    for lo, hi in zip(a_cut, a_cut[1:]):
                        if hi > lo:
                            nc.sync.dma_start(
                                out=xt[0:64, 1 + (lo - a_lo) : 1 + (hi - a_lo), :],
                                in_=x.ap()[b, :, 0, lo:hi, :],
                            )
                    # first block's B half rides the scalar HWDGE queue (in
                    # parallel with A on sync) for a fast start; afterwards
                    # scalar belongs to E-output, so B stays on sync to keep
                    # input ordered ahead of everything else there
                    beng = nc.scalar if (b == 0 and blk == 0) else nc.sync
                    for lo, hi in zip(b_cut, b_cut[1:]):
                        if hi > lo:
                            beng.dma_start(
                                out=xt[64:128, b_s + (lo - b_lo) : b_s + (hi - b_lo), :],
                                in_=x.ap()[b, :, 1, lo:hi, :],
                            )
                    for g in range(ngrp):
                        hg = (h0 + 16 * g) // 16
                        stE = spool.tile([128, 4, 2, W], f16, tag="stE")
                        stO = spool.tile([128, 4, 2, W], f16, tag="stO")
                        for uu in range(4):
                            r0 = R + 32 * g + 8 * uu
                            s0 = (r0 - R) // 2 + 1  # A-chunk of x row r0
                            psE = ppool.tile([128, 2, W], f32, tag="psE")
                            psO = ppool.tile([128, 2, W], f32, tag="psO")
                            # E mains: out rows (r0, r0+2 | r0+4, r0+6), taps kh=1,2
                            for kw in range(3):
                                st_ = kw == 0
                                nc.tensor.matmul(
                                    psE[0:64], wt[:, kw, :],
                                    xt[:, s0 : s0 + 2, kw : kw + W],
                                    start=st_, stop=False,
                                )
                                nc.tensor.matmul(
                                    psE[64:128], wt[:, kw, :],
                                    xt[:, s0 + 2 : s0 + 4, kw : kw + W],
                                    start=st_, stop=False,
                                )
                            # O mains: out rows (r0+1, r0+3 | r0+5, r0+7), taps kh=0,1
                            for kw in range(3):
                                st_ = kw == 0
                                nc.tensor.matmul(
                                    psO[0:64], wt[:, 3 + kw, :],
                                    xt[:, s0 : s0 + 2, kw : kw + W],
                                    start=st_, stop=False,
                                )
                                nc.tensor.matmul(
                                    psO[64:128], wt[:, 3 + kw, :],
                                    xt[:, s0 + 2 : s0 + 4, kw : kw + W],
                                    start=st_, stop=False,
                                )
                            # leftovers (4-way concurrent PE quadrants):
                            # E: tap kh=0 from half B; O: tap kh=2 from half A
                            for kw in range(3):
                                sp_ = kw == 2
                                nc.tensor.matmul(
                                    psE[0:64], wt[64:128, 6 + kw, :],
                                    xt[64:128, s0 - 1 : s0 + 1, kw : kw + W],
                                    start=False, stop=sp_,
                                )
                                nc.tensor.matmul(
                                    psE[64:128], wt[64:128, 6 + kw, :],
                                    xt[64:128, s0 + 1 : s0 + 3, kw : kw + W],
                                    start=False, stop=sp_,
                                )
                                nc.tensor.matmul(
                                    psO[0:64], wt[0:64, 6 + kw, :],
                                    xt[0:64, s0 + 1 : s0 + 3, kw : kw + W],
                                    start=False, stop=sp_,
                                )
                                nc.tensor.matmul(
                                    psO[64:128], wt[0:64, 6 + kw, :],
                                    xt[0:64, s0 + 3 : s0 + 5, kw : kw + W],
                                    start=False, stop=sp_,
                                )
                            nc.scalar.copy(stE[:, uu], psE[:])
                            nc.vector.tensor_copy(out=stO[:, uu], in_=psO[:])
                        # group output DMAs split across the scalar HWDGE
                        # queue and the gpsimd SWDGE queue; the sync queue
                        # carries only input so blocks never stall on output
                        nc.scalar.dma_start(out=outr[b, 0, hg], in_=stE[:])
                        nc.gpsimd.dma_start(out=outr[b, 1, hg], in_=stO[:])
    nc.compile()
    return nc


def normalize_weight(weight):
    """Host-side equalized-lr + demodulation of the [O,I,3,3] weight."""
    w = np.asarray(weight, dtype=np.float32) * np.float32(C_EQ)
    sigma_inv = 1.0 / np.sqrt(
        np.sum((w * w).astype(np.float32), axis=(1, 2, 3), keepdims=True) + EPS
    )
    return (w * sigma_inv.astype(np.float32)).astype(np.float32)


def pack_weights(w_norm):
    """Pack normalized [O,I,kh,kw] weights into the [128, 9, 64] SBUF image.

    Column group g = kw for the even-row mains (rows 0:64 <- kh=1,
    rows 64:128 <- kh=2), g = 3+kw for odd-row mains (kh=0 / kh=1),
    g = 6+kw for the leftovers (rows 0:64 <- kh=2, rows 64:128 <- kh=0).
    Each [64, 64] slice is wT = w[:, :, kh, kw].T (contraction dim first).
    """
    wt = np.transpose(w_norm, (2, 3, 1, 0))  # [kh, kw, in, out]
    wpack = np.zeros((128, 9, 64), dtype=np.float32)
    for kw in range(3):
        wpack[0:64, kw] = wt[1, kw]
        wpack[64:128, kw] = wt[2, kw]
        wpack[0:64, 3 + kw] = wt[0, kw]
        wpack[64:128, 3 + kw] = wt[1, kw]
        wpack[0:64, 6 + kw] = wt[2, kw]
        wpack[64:128, 6 + kw] = wt[0, kw]
    return wpack


_NC_CACHE = {}


def _get_nc(bpc, h, block=64):
    key = (bpc, h, block)
    if key not in _NC_CACHE:
        _NC_CACHE[key] = build_nc(bpc, h, block)
    return _NC_CACHE[key]


def split_parity(x_f32):
    """[b, c, h, w] f32 -> bf16 [b, c, 2, h//2, w+2]: row parity split plus
    zero border columns (p=0 even rows, p=1 odd rows)."""
    import ml_dtypes

    b, c, h, w = x_f32.shape
    xb = x_f32.astype(ml_dtypes.bfloat16)
    xP = np.zeros((b, c, 2, h // 2, w + 2), dtype=ml_dtypes.bfloat16)
    xP[:, :, 0, :, 1:-1] = xb[:, :, 0::2]
    xP[:, :, 1, :, 1:-1] = xb[:, :, 1::2]
    return xP


def merge_parity(outP):
    """Device [b, p, hg, ud, c, uu, up, w] -> fp32 [b, c, h, w] with
    h = 2*(16*hg + 4*uu + 2*ud + up) + p."""
    b, p, hg, ud, c, uu, up, w = outP.shape
    o = np.transpose(outP, (0, 4, 2, 5, 3, 6, 1, 7))  # b c hg uu ud up p w
    return np.ascontiguousarray(
        o.reshape(b, c, hg * uu * ud * up * p, w), dtype=np.float32
    )


def kernel(x, weight):
    import ml_dtypes
    from concourse import bass_utils

    x = np.asarray(x, dtype=np.float32)
    weight = np.asarray(weight, dtype=np.float32)
    assert x.shape == (B_FULL, IN_F, H_FULL, W), x.shape

    xP = split_parity(x)
    wpack = pack_weights(normalize_weight(weight)).astype(ml_dtypes.bfloat16)
    bpc = B_FULL // N_CORES
    nc = _get_nc(bpc, H_FULL)
    in_maps = [
        {"x": xP[i * bpc : (i + 1) * bpc], "wpack": wpack} for i in range(N_CORES)
    ]
    res = bass_utils.run_bass_kernel_spmd(nc, in_maps, core_ids=list(range(N_CORES)))
    return np.concatenate([merge_parity(r["out"]) for r in res.results], axis=0)


# revision 15
# speedup vs baseline: 1.0078x; 1.0049x over previous
"""Trainium2 Bass kernel for Conv2dWeightModulate (no style).

The reference computes an equalized-lr + demodulated 3x3 conv:
    w = weight * C_EQ;  w *= rsqrt(sum(w^2, (I,K,K)) + eps);  out = conv2d(x, w, pad=1)

The tiny weight normalization runs on host (numpy); the conv runs on 8
NeuronCores, data-parallel over the batch (2 images per core).

Host-side data layout: x is cast to bf16 and split by row parity into
xP[b, c, p, h2, w] (= x[b, c, 2*h2+p, w]) so every DMA reads long
contiguous spans; the device writes a parity-split fp16 output that the
host re-interleaves (and upcasts to fp32).

Device kernel layout (per core):
  x is stored in SBUF parity-interleaved: partitions 0-63 hold the 64
  channels of even image rows, partitions 64-127 the odd rows, with each
  row padded to 258 columns (zero borders give the conv its padding).
  Chunk column s of a block with row base R holds:
      half A (parts 0:64):   x row R + 2(s-1)
      half B (parts 64:128): x row R + 2s - 1
  so chunk s aligns x rows (2j, 2j+1) vertically.  A 3x3 conv then becomes,
  per pair of same-parity output rows (one 512-wide matmul free dim):
      - even rows: K=128 matmul (taps kh=1+kh=2) x3 kw  +  K=64 (kh=0) x3
      - odd rows:  K=128 matmul (taps kh=0+kh=1) x3 kw  +  K=64 (kh=2) x3
  Adjacent row-pairs are col-tiled (tile_position via PSUM base partition
  64) so the pair runs concurrently on disjoint PE column groups; the
  K=64 leftovers of even/odd chunks land on disjoint PE quadrants and run
  4-way concurrent.  Each even/odd PSUM pair shares one full-width
  [128, 2, W] PSUM tile (halves on disjoint partition ranges), so copies
  are full-128-lane and only 2 PSUM banks are live per row-group,
  letting the pool double-buffer 4 deep.  Accumulation is fp32 in PSUM;
  outputs staged through SBUF as fp16 in 32-row groups and DMAed out.
"""

import numpy as np

IN_F = 64
OUT_F = 64
KS = 3
EPS = 1e-05
C_EQ = 1.0 / np.sqrt(IN_F * KS * KS)

B_FULL = 16
H_FULL = 256
W = 256
N_CORES = 8
CW = W + 2  # padded row width


def build_nc(bpc, h, block=64):
    """Build the per-core Bass program: bpc images of [64, h, 256] each."""
    from concourse import bacc
    import concourse.mybir as mybir
    from concourse.tile import TileContext

    assert h % block == 0 and block % 32 == 0
    nblk = h // block
    ngrp = block // 32  # 32-row output staging groups per block
    sch = block // 2 + 2  # chunk columns per x tile
    f32 = mybir.dt.float32
    bf16 = mybir.dt.bfloat16
    f16 = mybir.dt.float16

    nc = bacc.Bacc("TRN2", target_bir_lowering=False, debug=False)
    x = nc.dram_tensor("x", [bpc, IN_F, 2, h // 2, CW], bf16, kind="ExternalInput")
    wp = nc.dram_tensor("wpack", [128, 9, 64], bf16, kind="ExternalInput")
    # output stays in staging order: h = 2*(16*hg + 4*uu + 2*ud + up) + p,
    # so each 32-row group+parity is one fully contiguous 512KB DMA
    # (partition-major, 4KB contiguous per partition); host reassembles.
    # A staging tile's partition half selects ud, its inner row dim is up.
    out = nc.dram_tensor(
        "out", [bpc, 2, h // 32, 2, OUT_F, 4, 2, W], f16, kind="ExternalOutput"
    )
    outr = out.ap().rearrange("b p hg ud c uu up w -> b p hg (ud c) uu up w")

    with TileContext(nc) as tc:
        with (
            tc.tile_pool(name="xp", bufs=5) as xpool,
            tc.tile_pool(name="wpool", bufs=1) as wpool,
            tc.tile_pool(name="st", bufs=8) as spool,
            tc.tile_pool(name="ps", bufs=4, space="PSUM") as ppool,
        ):
            wt = wpool.tile([128, 9, 64], bf16)
            # weights lead the scalar HWDGE queue; the sync HWDGE queue
            # starts on the first x tile in parallel
            nc.scalar.dma_start(out=wt[:], in_=wp.ap())
            pending = None  # deferred E-output DMA (dest_ap, tile)
            for b in range(bpc):
                for blk in range(nblk):
                    R = blk * block
                    h0 = R // 2
                    xt = xpool.tile([128, sch, CW], bf16, tag="xt")
                    # host pre-pads rows to 258 with zero borders, so every
                    # transfer is one contiguous span per channel
                    # half A <- even x rows R .. R+block (chunks 1..sch-1)
                    # half B <- odd x rows R-1 .. R+block-1 (chunks 0..sch-2)
                    if blk == nblk - 1:
                        a_lo, a_hi = h0, h0 + sch - 2
                        nc.gpsimd.memset(xt[0:64, sch - 1, :], 0.0)
                    else:
                        a_lo, a_hi = h0, h0 + sch - 1
                    if blk == 0:
                        nc.gpsimd.memset(xt[64:128, 0, :], 0.0)
                        b_s, b_lo, b_hi = 1, 0, sch - 2
                    else:
                        b_s, b_lo, b_hi = 0, h0 - 1, h0 + sch - 2
                    if b == 0 and blk == 0:
                        # tiny leading pieces so the first row-group's matmuls
                        # gate on ~400KB instead of the whole 1.1MB tile
                        cuts = [6, 14]
                    else:
                        cuts = [sch // 2]
                    a_cut = [a_lo] + [min(a_lo + c, a_hi) for c in cuts] + [a_hi]
                    b_cut = [b_lo] + [min(b_lo + c, b_hi) for c in cuts] + [b_hi]
                    for lo, hi in zip(a_cut, a_cut[1:]):
                        if hi > lo:
                            nc.sync.dma_start(
                                out=xt[0:64, 1 + (lo - a_lo) : 1 + (hi - a_lo), :],
                                in_=x.ap()[b, :, 0, lo:hi, :],
                            )
                    # first block's B half rides the scalar HWDGE queue (in
                    # parallel with A on sync) for a fast start; afterwards
                    # scalar belongs to E-output, so B stays on sync to keep
                    # input ordered ahead of everything else there
                    beng = nc.scalar if (b == 0 and blk == 0) else nc.sync
                    for lo, hi in zip(b_cut, b_cut[1:]):
                        if hi > lo:
                            beng.dma_start(
                                out=xt[64:128, b_s + (lo - b_lo) : b_s + (hi - b_lo), :],
                                in_=x.ap()[b, :, 1, lo:hi, :],
                            )
                    for g in range(ngrp):
                        hg = (h0 + 16 * g) // 16
                        stE = spool.tile([128, 4, 2, W], f16, tag="stE")
                        stO = spool.tile([128, 4, 2, W], f16, tag="stO")
                        for uu in range(4):
                            if uu == 2 and pending is not None:
                                # previous group's E DMA, issued only now so
                                # its copy-completion wait is pre-satisfied
                                # and never FIFO-blocks the scalar engine's
                                # own copy stream
                                nc.scalar.dma_start(out=pending[0], in_=pending[1][:])
                                pending = None
                            r0 = R + 32 * g + 8 * uu
                            s0 = (r0 - R) // 2 + 1  # A-chunk of x row r0
                            psE = ppool.tile([128, 2, W], f32, tag="psE")
                            psO = ppool.tile([128, 2, W], f32, tag="psO")
                            # E mains: out rows (r0, r0+2 | r0+4, r0+6), taps kh=1,2
                            for kw in range(3):
                                st_ = kw == 0
                                nc.tensor.matmul(
                                    psE[0:64], wt[:, kw, :],
                                    xt[:, s0 : s0 + 2, kw : kw + W],
                                    start=st_, stop=False,
                                )
                                nc.tensor.matmul(
                                    psE[64:128], wt[:, kw, :],
                                    xt[:, s0 + 2 : s0 + 4, kw : kw + W],
                                    start=st_, stop=False,
                                )
                            # O mains: out rows (r0+1, r0+3 | r0+5, r0+7), taps kh=0,1
                            for kw in range(3):
                                st_ = kw == 0
                                nc.tensor.matmul(
                                    psO[0:64], wt[:, 3 + kw, :],
                                    xt[:, s0 : s0 + 2, kw : kw + W],
                                    start=st_, stop=False,
                                )
                                nc.tensor.matmul(
                                    psO[64:128], wt[:, 3 + kw, :],
                                    xt[:, s0 + 2 : s0 + 4, kw : kw + W],
                                    start=st_, stop=False,
                                )
                            # leftovers (4-way concurrent PE quadrants):
                            # E: tap kh=0 from half B; O: tap kh=2 from half A
                            for kw in range(3):
                                sp_ = kw == 2
                                nc.tensor.matmul(
                                    psE[0:64], wt[64:128, 6 + kw, :],
                                    xt[64:128, s0 - 1 : s0 + 1, kw : kw + W],
                                    start=False, stop=sp_,
                                )
                                nc.tensor.matmul(
                                    psE[64:128], wt[64:128, 6 + kw, :],
                                    xt[64:128, s0 + 1 : s0 + 3, kw : kw + W],
                                    start=False, stop=sp_,
                                )
                                nc.tensor.matmul(
                                    psO[0:64], wt[0:64, 6 + kw, :],
                                    xt[0:64, s0 + 1 : s0 + 3, kw : kw + W],
                                    start=False, stop=sp_,
                                )
                                nc.tensor.matmul(
                                    psO[64:128], wt[0:64, 6 + kw, :],
                                    xt[0:64, s0 + 3 : s0 + 5, kw : kw + W],
                                    start=False, stop=sp_,
                                )
                            nc.scalar.copy(stE[:, uu], psE[:])
                            nc.vector.tensor_copy(out=stO[:, uu], in_=psO[:])
                        # group output DMAs split across the scalar HWDGE
                        # queue (E, deferred) and the gpsimd SWDGE queue (O);
                        # the sync queue carries only input so blocks never
                        # stall on output
                        pending = (outr[b, 0, hg], stE)
                        nc.gpsimd.dma_start(out=outr[b, 1, hg], in_=stO[:])
            if pending is not None:
                nc.scalar.dma_start(out=pending[0], in_=pending[1][:])
    nc.compile()
    return nc


def normalize_weight(weight):
    """Host-side equalized-lr + demodulation of the [O,I,3,3] weight."""
    w = np.asarray(weight, dtype=np.float32) * np.float32(C_EQ)
    sigma_inv = 1.0 / np.sqrt(
        np.sum((w * w).astype(np.float32), axis=(1, 2, 3), keepdims=True) + EPS
    )
    return (w * sigma_inv.astype(np.float32)).astype(np.float32)


def pack_weights(w_norm):
    """Pack normalized [O,I,kh,kw] weights into the [128, 9, 64] SBUF image.

    Column group g = kw for the even-row mains (rows 0:64 <- kh=1,
    rows 64:128 <- kh=2), g = 3+kw for odd-row mains (kh=0 / kh=1),
    g = 6+kw for the leftovers (rows 0:64 <- kh=2, rows 64:128 <- kh=0).
    Each [64, 64] slice is wT = w[:, :, kh, kw].T (contraction dim first).
    """
    wt = np.transpose(w_norm, (2, 3, 1, 0))  # [kh, kw, in, out]
    wpack = np.zeros((128, 9, 64), dtype=np.float32)
    for kw in range(3):
        wpack[0:64, kw] = wt[1, kw]
        wpack[64:128, kw] = wt[2, kw]
        wpack[0:64, 3 + kw] = wt[0, kw]
        wpack[64:128, 3 + kw] = wt[1, kw]
        wpack[0:64, 6 + kw] = wt[2, kw]
        wpack[64:128, 6 + kw] = wt[0, kw]
    return wpack


_NC_CACHE = {}


def _get_nc(bpc, h, block=64):
    key = (bpc, h, block)
    if key not in _NC_CACHE:
        _NC_CACHE[key] = build_nc(bpc, h, block)
    return _NC_CACHE[key]


def split_parity(x_f32):
    """[b, c, h, w] f32 -> bf16 [b, c, 2, h//2, w+2]: row parity split plus
    zero border columns (p=0 even rows, p=1 odd rows)."""
    import ml_dtypes

    b, c, h, w = x_f32.shape
    xb = x_f32.astype(ml_dtypes.bfloat16)
    xP = np.zeros((b, c, 2, h // 2, w + 2), dtype=ml_dtypes.bfloat16)
    xP[:, :, 0, :, 1:-1] = xb[:, :, 0::2]
    xP[:, :, 1, :, 1:-1] = xb[:, :, 1::2]
    return xP


def merge_parity(outP):
    """Device [b, p, hg, ud, c, uu, up, w] -> fp32 [b, c, h, w] with
    h = 2*(16*hg + 4*uu + 2*ud + up) + p."""
    b, p, hg, ud, c, uu, up, w = outP.shape
    o = np.transpose(outP, (0, 4, 2, 5, 3, 6, 1, 7))  # b c hg uu ud up p w
    return np.ascontiguousarray(
        o.reshape(b, c, hg * uu * ud * up * p, w), dtype=np.float32
    )


def kernel(x, weight):
    import ml_dtypes
    from concourse import bass_utils

    x = np.asarray(x, dtype=np.float32)
    weight = np.asarray(weight, dtype=np.float32)
    assert x.shape == (B_FULL, IN_F, H_FULL, W), x.shape

    xP = split_parity(x)
    wpack = pack_weights(normalize_weight(weight)).astype(ml_dtypes.bfloat16)
    bpc = B_FULL // N_CORES
    nc = _get_nc(bpc, H_FULL)
    in_maps = [
        {"x": xP[i * bpc : (i + 1) * bpc], "wpack": wpack} for i in range(N_CORES)
    ]
    res = bass_utils.run_bass_kernel_spmd(nc, in_maps, core_ids=list(range(N_CORES)))
    return np.concatenate([merge_parity(r["out"]) for r in res.results], axis=0)


# revision 19
# speedup vs baseline: 1.0244x; 1.0165x over previous
"""Trainium2 Bass kernel for Conv2dWeightModulate (no style).

The reference computes an equalized-lr + demodulated 3x3 conv:
    w = weight * C_EQ;  w *= rsqrt(sum(w^2, (I,K,K)) + eps);  out = conv2d(x, w, pad=1)

The tiny weight normalization runs on host (numpy); the conv runs on 8
NeuronCores, data-parallel over the batch (2 images per core).

Host-side data layout: x is cast to bf16 and split by row parity into
xP[b, c, p, h2, w] (= x[b, c, 2*h2+p, w]) so every DMA reads long
contiguous spans; the device writes a parity-split fp16 output that the
host re-interleaves (and upcasts to fp32).

Device kernel layout (per core):
  x is stored in SBUF parity-interleaved: partitions 0-63 hold the 64
  channels of even image rows, partitions 64-127 the odd rows, with each
  row padded to 258 columns (zero borders give the conv its padding).
  Chunk column s of a block with row base R holds:
      half A (parts 0:64):   x row R + 2(s-1)
      half B (parts 64:128): x row R + 2s - 1
  so chunk s aligns x rows (2j, 2j+1) vertically.  A 3x3 conv then becomes,
  per pair of same-parity output rows (one 512-wide matmul free dim):
      - even rows: K=128 matmul (taps kh=1+kh=2) x3 kw  +  K=64 (kh=0) x3
      - odd rows:  K=128 matmul (taps kh=0+kh=1) x3 kw  +  K=64 (kh=2) x3
  Adjacent row-pairs are col-tiled (tile_position via PSUM base partition
  64) so the pair runs concurrently on disjoint PE column groups; the
  K=64 leftovers of even/odd chunks land on disjoint PE quadrants and run
  4-way concurrent.  Each even/odd PSUM pair shares one full-width
  [128, 2, W] PSUM tile (halves on disjoint partition ranges), so copies
  are full-128-lane and only 2 PSUM banks are live per row-group,
  letting the pool double-buffer 4 deep.  Accumulation is fp32 in PSUM;
  outputs staged through SBUF as fp16 in 32-row groups and DMAed out.
"""

import numpy as np

IN_F = 64
OUT_F = 64
KS = 3
EPS = 1e-05
C_EQ = 1.0 / np.sqrt(IN_F * KS * KS)

B_FULL = 16
H_FULL = 256
W = 256
N_CORES = 8
CW = W + 2  # padded row width


def build_nc(bpc, h, block=64):
    """Build the per-core Bass program: bpc images of [64, h, 256] each."""
    from concourse import bacc
    import concourse.mybir as mybir
    from concourse.tile import TileContext

    assert h % block == 0 and block % 32 == 0
    nblk = h // block
    ngrp = block // 32  # 32-row output staging groups per block
    sch = block // 2 + 2  # chunk columns per x tile
    f32 = mybir.dt.float32
    bf16 = mybir.dt.bfloat16
    f16 = mybir.dt.float16

    nc = bacc.Bacc("TRN2", target_bir_lowering=False, debug=False)
    x = nc.dram_tensor("x", [bpc, IN_F, 2, h // 2, CW], bf16, kind="ExternalInput")
    wp = nc.dram_tensor("wpack", [128, 9, 64], bf16, kind="ExternalInput")
    # output stays in staging order: h = 2*(16*hg + 4*uu + 2*ud + up) + p,
    # so each 32-row group+parity is one fully contiguous 512KB DMA
    # (partition-major, 4KB contiguous per partition); host reassembles.
    # A staging tile's partition half selects ud, its inner row dim is up.
    out = nc.dram_tensor(
        "out", [bpc, 2, h // 32, 2, OUT_F, 4, 2, W], f16, kind="ExternalOutput"
    )
    outr = out.ap().rearrange("b p hg ud c uu up w -> b p hg (ud c) uu up w")

    with TileContext(nc) as tc:
        with (
            tc.tile_pool(name="xp", bufs=5) as xpool,
            tc.tile_pool(name="wpool", bufs=1) as wpool,
            tc.tile_pool(name="st", bufs=10) as spool,
            tc.tile_pool(name="ps", bufs=4, space="PSUM") as ppool,
        ):
            wt = wpool.tile([128, 9, 64], bf16)
            # weights lead the scalar HWDGE queue; the sync HWDGE queue
            # starts on the first x tile in parallel
            nc.scalar.dma_start(out=wt[:], in_=wp.ap())
            for b in range(bpc):
                for blk in range(nblk):
                    R = blk * block
                    h0 = R // 2
                    xt = xpool.tile([128, sch, CW], bf16, tag="xt")
                    # host pre-pads rows to 258 with zero borders, so every
                    # transfer is one contiguous span per channel
                    # half A <- even x rows R .. R+block (chunks 1..sch-1)
                    # half B <- odd x rows R-1 .. R+block-1 (chunks 0..sch-2)
                    if blk == nblk - 1:
                        a_lo, a_hi = h0, h0 + sch - 2
                        nc.gpsimd.memset(xt[0:64, sch - 1, :], 0.0)
                    else:
                        a_lo, a_hi = h0, h0 + sch - 1
                    if blk == 0:
                        nc.gpsimd.memset(xt[64:128, 0, :], 0.0)
                        b_s, b_lo, b_hi = 1, 0, sch - 2
                    else:
                        b_s, b_lo, b_hi = 0, h0 - 1, h0 + sch - 2
                    if b == 0 and blk == 0:
                        # tiny leading pieces so the first row-group's matmuls
                        # gate on ~400KB instead of the whole 1.1MB tile
                        cuts = [6, 14]
                    else:
                        cuts = [sch // 2]
                    a_cut = [a_lo] + [min(a_lo + c, a_hi) for c in cuts] + [a_hi]
                    b_cut = [b_lo] + [min(b_lo + c, b_hi) for c in cuts] + [b_hi]
                    for lo, hi in zip(a_cut, a_cut[1:]):
                        if hi > lo:
                            nc.sync.dma_start(
                                out=xt[0:64, 1 + (lo - a_lo) : 1 + (hi - a_lo), :],
                                in_=x.ap()[b, :, 0, lo:hi, :],
                            )
                    # first block's B half rides the scalar HWDGE queue (in
                    # parallel with A on sync) for a fast start; afterwards
                    # scalar belongs to E-output, so B stays on sync to keep
                    # input ordered ahead of everything else there
                    beng = nc.scalar if (b == 0 and blk == 0) else nc.sync
                    for lo, hi in zip(b_cut, b_cut[1:]):
                        if hi > lo:
                            beng.dma_start(
                                out=xt[64:128, b_s + (lo - b_lo) : b_s + (hi - b_lo), :],
                                in_=x.ap()[b, :, 1, lo:hi, :],
                            )
                    for g in range(ngrp):
                        hg = (h0 + 16 * g) // 16
                        stE = spool.tile([128, 4, 2, W], f16, tag="stE")
                        stO = spool.tile([128, 4, 2, W], f16, tag="stO")
                        for uu in range(4):
                            r0 = R + 32 * g + 8 * uu
                            s0 = (r0 - R) // 2 + 1  # A-chunk of x row r0
                            psE = ppool.tile([128, 2, W], f32, tag="psE")
                            psO = ppool.tile([128, 2, W], f32, tag="psO")
                            # E mains: out rows (r0, r0+2 | r0+4, r0+6), taps kh=1,2
                            for kw in range(3):
                                st_ = kw == 0
                                nc.tensor.matmul(
                                    psE[0:64], wt[:, kw, :],
                                    xt[:, s0 : s0 + 2, kw : kw + W],
                                    start=st_, stop=False,
                                )
                                nc.tensor.matmul(
                                    psE[64:128], wt[:, kw, :],
                                    xt[:, s0 + 2 : s0 + 4, kw : kw + W],
                                    start=st_, stop=False,
                                )
                            # O mains: out rows (r0+1, r0+3 | r0+5, r0+7), taps kh=0,1
                            for kw in range(3):
                                st_ = kw == 0
                                nc.tensor.matmul(
                                    psO[0:64], wt[:, 3 + kw, :],
                                    xt[:, s0 : s0 + 2, kw : kw + W],
                                    start=st_, stop=False,
                                )
                                nc.tensor.matmul(
                                    psO[64:128], wt[:, 3 + kw, :],
                                    xt[:, s0 + 2 : s0 + 4, kw : kw + W],
                                    start=st_, stop=False,
                                )
                            # leftovers (4-way concurrent PE quadrants):
                            # E: tap kh=0 from half B; O: tap kh=2 from half A
                            for kw in range(3):
                                sp_ = kw == 2
                                nc.tensor.matmul(
                                    psE[0:64], wt[64:128, 6 + kw, :],
                                    xt[64:128, s0 - 1 : s0 + 1, kw : kw + W],
                                    start=False, stop=sp_,
                                )
                                nc.tensor.matmul(
                                    psE[64:128], wt[64:128, 6 + kw, :],
                                    xt[64:128, s0 + 1 : s0 + 3, kw : kw + W],
                                    start=False, stop=sp_,
                                )
                                nc.tensor.matmul(
                                    psO[0:64], wt[0:64, 6 + kw, :],
                                    xt[0:64, s0 + 1 : s0 + 3, kw : kw + W],
                                    start=False, stop=sp_,
                                )
                                nc.tensor.matmul(
                                    psO[64:128], wt[0:64, 6 + kw, :],
                                    xt[0:64, s0 + 3 : s0 + 5, kw : kw + W],
                                    start=False, stop=sp_,
                                )
                            nc.scalar.copy(stE[:, uu], psE[:])
                            nc.vector.tensor_copy(out=stO[:, uu], in_=psO[:])
                        # output DMAs ride the gpsimd SWDGE queue: its FIFO
                        # carries nothing matmul-critical, so completion
                        # guards never stall compute (the copy engines must
                        # not host these — their FIFOs would convoy).  Once
                        # input is done the sync HWDGE queue is free, so the
                        # final blocks' E groups drain there for a fast tail.
                        if b == bpc - 1 and blk >= nblk - 2:
                            nc.sync.dma_start(out=outr[b, 0, hg], in_=stE[:])
                        else:
                            nc.gpsimd.dma_start(out=outr[b, 0, hg], in_=stE[:])
                        nc.gpsimd.dma_start(out=outr[b, 1, hg], in_=stO[:])
    nc.compile()
    return nc


def normalize_weight(weight):
    """Host-side equalized-lr + demodulation of the [O,I,3,3] weight."""
    w = np.asarray(weight, dtype=np.float32) * np.float32(C_EQ)
    sigma_inv = 1.0 / np.sqrt(
        np.sum((w * w).astype(np.float32), axis=(1, 2, 3), keepdims=True) + EPS
    )
    return (w * sigma_inv.astype(np.float32)).astype(np.float32)


def pack_weights(w_norm):
    """Pack normalized [O,I,kh,kw] weights into the [128, 9, 64] SBUF image.

    Column group g = kw for the even-row mains (rows 0:64 <- kh=1,
    rows 64:128 <- kh=2), g = 3+kw for odd-row mains (kh=0 / kh=1),
    g = 6+kw for the leftovers (rows 0:64 <- kh=2, rows 64:128 <- kh=0).
    Each [64, 64] slice is wT = w[:, :, kh, kw].T (contraction dim first).
    """
    wt = np.transpose(w_norm, (2, 3, 1, 0))  # [kh, kw, in, out]
    wpack = np.zeros((128, 9, 64), dtype=np.float32)
    for kw in range(3):
        wpack[0:64, kw] = wt[1, kw]
        wpack[64:128, kw] = wt[2, kw]
        wpack[0:64, 3 + kw] = wt[0, kw]
        wpack[64:128, 3 + kw] = wt[1, kw]
        wpack[0:64, 6 + kw] = wt[2, kw]
        wpack[64:128, 6 + kw] = wt[0, kw]
    return wpack


_NC_CACHE = {}


def _get_nc(bpc, h, block=64):
    key = (bpc, h, block)
    if key not in _NC_CACHE:
        _NC_CACHE[key] = build_nc(bpc, h, block)
    return _NC_CACHE[key]


def split_parity(x_f32):
    """[b, c, h, w] f32 -> bf16 [b, c, 2, h//2, w+2]: row parity split plus
    zero border columns (p=0 even rows, p=1 odd rows)."""
    import ml_dtypes

    b, c, h, w = x_f32.shape
    xb = x_f32.astype(ml_dtypes.bfloat16)
    xP = np.zeros((b, c, 2, h // 2, w + 2), dtype=ml_dtypes.bfloat16)
    xP[:, :, 0, :, 1:-1] = xb[:, :, 0::2]
    xP[:, :, 1, :, 1:-1] = xb[:, :, 1::2]
    return xP


def merge_parity(outP):
    """Device [b, p, hg, ud, c, uu, up, w] -> fp32 [b, c, h, w] with
    h = 2*(16*hg + 4*uu + 2*ud + up) + p."""
    b, p, hg, ud, c, uu, up, w = outP.shape
    o = np.transpose(outP, (0, 4, 2, 5, 3, 6, 1, 7))  # b c hg uu ud up p w
    return np.ascontiguousarray(
        o.reshape(b, c, hg * uu * ud * up * p, w), dtype=np.float32
    )


def kernel(x, weight):
    import ml_dtypes
    from concourse import bass_utils

    x = np.asarray(x, dtype=np.float32)
    weight = np.asarray(weight, dtype=np.float32)
    assert x.shape == (B_FULL, IN_F, H_FULL, W), x.shape

    xP = split_parity(x)
    wpack = pack_weights(normalize_weight(weight)).astype(ml_dtypes.bfloat16)
    bpc = B_FULL // N_CORES
    nc = _get_nc(bpc, H_FULL)
    in_maps = [
        {"x": xP[i * bpc : (i + 1) * bpc], "wpack": wpack} for i in range(N_CORES)
    ]
    res = bass_utils.run_bass_kernel_spmd(nc, in_maps, core_ids=list(range(N_CORES)))
    return np.concatenate([merge_parity(r["out"]) for r in res.results], axis=0)


# revision 27
# speedup vs baseline: 1.0428x; 1.0180x over previous
"""Trainium2 Bass kernel for Conv2dWeightModulate (no style).

The reference computes an equalized-lr + demodulated 3x3 conv:
    w = weight * C_EQ;  w *= rsqrt(sum(w^2, (I,K,K)) + eps);  out = conv2d(x, w, pad=1)

The tiny weight normalization runs on host (numpy); the conv runs on 8
NeuronCores, data-parallel over the batch (2 images per core).

Host-side data layout: x is cast to bf16 and split by row parity into
xP[b, c, p, h2, w] (= x[b, c, 2*h2+p, w]) so every DMA reads long
contiguous spans; the device writes a parity-split fp16 output that the
host re-interleaves (and upcasts to fp32).

Device kernel layout (per core):
  x is stored in SBUF parity-interleaved: partitions 0-63 hold the 64
  channels of even image rows, partitions 64-127 the odd rows, with each
  row padded to 258 columns (zero borders give the conv its padding).
  Chunk column s of a block with row base R holds:
      half A (parts 0:64):   x row R + 2(s-1)
      half B (parts 64:128): x row R + 2s - 1
  so chunk s aligns x rows (2j, 2j+1) vertically.  A 3x3 conv then becomes,
  per pair of same-parity output rows (one 512-wide matmul free dim):
      - even rows: K=128 matmul (taps kh=1+kh=2) x3 kw  +  K=64 (kh=0) x3
      - odd rows:  K=128 matmul (taps kh=0+kh=1) x3 kw  +  K=64 (kh=2) x3
  Adjacent row-pairs are col-tiled (tile_position via PSUM base partition
  64) so the pair runs concurrently on disjoint PE column groups; the
  K=64 leftovers of even/odd chunks land on disjoint PE quadrants and run
  4-way concurrent.  Each even/odd PSUM pair shares one full-width
  [128, 2, W] PSUM tile (halves on disjoint partition ranges), so copies
  are full-128-lane and only 2 PSUM banks are live per row-group,
  letting the pool double-buffer 4 deep.  Accumulation is fp32 in PSUM;
  outputs staged through SBUF as fp16 in 32-row groups and DMAed out.
"""

import numpy as np

IN_F = 64
OUT_F = 64
KS = 3
EPS = 1e-05
C_EQ = 1.0 / np.sqrt(IN_F * KS * KS)

B_FULL = 16
H_FULL = 256
W = 256
N_CORES = 8
CW = W + 2  # padded row width


def build_nc(bpc, h, block=64):
    """Build the per-core Bass program: bpc images of [64, h, 256] each."""
    from concourse import bacc
    import concourse.mybir as mybir
    from concourse.tile import TileContext

    assert h % block == 0 and block % 32 == 0
    nblk = h // block
    ngrp = block // 32  # 32-row output staging groups per block
    sch = block // 2 + 2  # chunk columns per x tile
    f32 = mybir.dt.float32
    bf16 = mybir.dt.bfloat16
    f16 = mybir.dt.float16

    nc = bacc.Bacc("TRN2", target_bir_lowering=False, debug=False)
    x = nc.dram_tensor("x", [bpc, IN_F, 2, h // 2, CW], bf16, kind="ExternalInput")
    wp = nc.dram_tensor("wpack", [128, 9, 64], bf16, kind="ExternalInput")
    # output stays in staging order: h = 2*(16*hg + 4*uu + 2*ud + up) + p,
    # so each 32-row group+parity is one fully contiguous 512KB DMA
    # (partition-major, 4KB contiguous per partition); host reassembles.
    # A staging tile's partition half selects ud, its inner row dim is up.
    out = nc.dram_tensor(
        "out", [bpc, 2, h // 32, 2, OUT_F, 4, 2, W], f16, kind="ExternalOutput"
    )
    outr = out.ap().rearrange("b p hg ud c uu up w -> b p hg (ud c) uu up w")

    with TileContext(nc) as tc:
        with (
            tc.tile_pool(name="xp", bufs=5) as xpool,
            tc.tile_pool(name="wpool", bufs=1) as wpool,
            tc.tile_pool(name="st", bufs=10) as spool,
            tc.tile_pool(name="ps", bufs=4, space="PSUM") as ppool,
        ):
            wt = wpool.tile([128, 9, 64], bf16)
            # weights lead the scalar HWDGE queue; the sync HWDGE queue
            # starts on the first x tile in parallel
            nc.scalar.dma_start(out=wt[:], in_=wp.ap())
            for b in range(bpc):
                for blk in range(nblk):
                    R = blk * block
                    h0 = R // 2
                    xt = xpool.tile([128, sch, CW], bf16, tag="xt")
                    # host pre-pads rows to 258 with zero borders, so every
                    # transfer is one contiguous span per channel
                    # half A <- even x rows R .. R+block (chunks 1..sch-1)
                    # half B <- odd x rows R-1 .. R+block-1 (chunks 0..sch-2)
                    if blk == nblk - 1:
                        a_lo, a_hi = h0, h0 + sch - 2
                        nc.gpsimd.memset(xt[0:64, sch - 1, :], 0.0)
                    else:
                        a_lo, a_hi = h0, h0 + sch - 1
                    if blk == 0:
                        nc.gpsimd.memset(xt[64:128, 0, :], 0.0)
                        b_s, b_lo, b_hi = 1, 0, sch - 2
                    else:
                        b_s, b_lo, b_hi = 0, h0 - 1, h0 + sch - 2
                    if b == 0 and blk == 0:
                        # tiny leading pieces: the very first matmuls gate on
                        # ~130KB (2 rows/half), the rest ladders in behind
                        cuts = [2, 6, 14]
                    else:
                        cuts = [sch // 2]
                    a_cut = [a_lo] + [min(a_lo + c, a_hi) for c in cuts] + [a_hi]
                    b_cut = [b_lo] + [min(b_lo + c, b_hi) for c in cuts] + [b_hi]
                    for lo, hi in zip(a_cut, a_cut[1:]):
                        if hi > lo:
                            nc.sync.dma_start(
                                out=xt[0:64, 1 + (lo - a_lo) : 1 + (hi - a_lo), :],
                                in_=x.ap()[b, :, 0, lo:hi, :],
                            )
                    # first block's B half rides the scalar HWDGE queue (in
                    # parallel with A on sync) for a fast start; afterwards
                    # scalar belongs to E-output, so B stays on sync to keep
                    # input ordered ahead of everything else there
                    beng = nc.scalar if (b == 0 and blk == 0) else nc.sync
                    for lo, hi in zip(b_cut, b_cut[1:]):
                        if hi > lo:
                            beng.dma_start(
                                out=xt[64:128, b_s + (lo - b_lo) : b_s + (hi - b_lo), :],
                                in_=x.ap()[b, :, 1, lo:hi, :],
                            )
                    for g in range(ngrp):
                        hg = (h0 + 16 * g) // 16
                        stE = spool.tile([128, 4, 2, W], f16, tag="stE")
                        stO = spool.tile([128, 4, 2, W], f16, tag="stO")
                        for uu in range(4):
                            r0 = R + 32 * g + 8 * uu
                            s0 = (r0 - R) // 2 + 1  # A-chunk of x row r0
                            psE = ppool.tile([128, 2, W], f32, tag="psE")
                            psO = ppool.tile([128, 2, W], f32, tag="psO")
                            # E mains: out rows (r0, r0+2 | r0+4, r0+6), taps kh=1,2
                            for kw in range(3):
                                st_ = kw == 0
                                nc.tensor.matmul(
                                    psE[0:64], wt[:, kw, :],
                                    xt[:, s0 : s0 + 2, kw : kw + W],
                                    start=st_, stop=False,
                                )
                                nc.tensor.matmul(
                                    psE[64:128], wt[:, kw, :],
                                    xt[:, s0 + 2 : s0 + 4, kw : kw + W],
                                    start=st_, stop=False,
                                )
                            # O mains: out rows (r0+1, r0+3 | r0+5, r0+7), taps kh=0,1
                            for kw in range(3):
                                st_ = kw == 0
                                nc.tensor.matmul(
                                    psO[0:64], wt[:, 3 + kw, :],
                                    xt[:, s0 : s0 + 2, kw : kw + W],
                                    start=st_, stop=False,
                                )
                                nc.tensor.matmul(
                                    psO[64:128], wt[:, 3 + kw, :],
                                    xt[:, s0 + 2 : s0 + 4, kw : kw + W],
                                    start=st_, stop=False,
                                )
                            # leftovers (4-way concurrent PE quadrants):
                            # E: tap kh=0 from half B; O: tap kh=2 from half A
                            for kw in range(3):
                                sp_ = kw == 2
                                nc.tensor.matmul(
                                    psE[0:64], wt[64:128, 6 + kw, :],
                                    xt[64:128, s0 - 1 : s0 + 1, kw : kw + W],
                                    start=False, stop=sp_,
                                )
                                nc.tensor.matmul(
                                    psE[64:128], wt[64:128, 6 + kw, :],
                                    xt[64:128, s0 + 1 : s0 + 3, kw : kw + W],
                                    start=False, stop=sp_,
                                )
                                nc.tensor.matmul(
                                    psO[0:64], wt[0:64, 6 + kw, :],
                                    xt[0:64, s0 + 1 : s0 + 3, kw : kw + W],
                                    start=False, stop=sp_,
                                )
                                nc.tensor.matmul(
                                    psO[64:128], wt[0:64, 6 + kw, :],
                                    xt[0:64, s0 + 3 : s0 + 5, kw : kw + W],
                                    start=False, stop=sp_,
                                )
                            nc.scalar.copy(stE[:, uu], psE[:])
                            nc.vector.tensor_copy(out=stO[:, uu], in_=psO[:])
                        # output DMAs ride the gpsimd SWDGE queue: its FIFO
                        # carries nothing matmul-critical, so completion
                        # guards never stall compute (the copy engines must
                        # not host these — their FIFOs would convoy).  Once
                        # input is done the sync HWDGE queue is free, so the
                        # final blocks' E groups drain there for a fast tail.
                        last_grp = b == bpc - 1 and blk == nblk - 1 and g == ngrp - 1
                        if b == bpc - 1 and blk >= nblk - 2:
                            nc.sync.dma_start(out=outr[b, 0, hg], in_=stE[:])
                        else:
                            nc.gpsimd.dma_start(out=outr[b, 0, hg], in_=stE[:])
                        if last_grp:
                            # final group: no copies remain behind this issue,
                            # so the fast scalar HWDGE queue is convoy-safe
                            nc.scalar.dma_start(out=outr[b, 1, hg], in_=stO[:])
                        else:
                            nc.gpsimd.dma_start(out=outr[b, 1, hg], in_=stO[:])
    nc.compile()
    return nc


def normalize_weight(weight):
    """Host-side equalized-lr + demodulation of the [O,I,3,3] weight."""
    w = np.asarray(weight, dtype=np.float32) * np.float32(C_EQ)
    sigma_inv = 1.0 / np.sqrt(
        np.sum((w * w).astype(np.float32), axis=(1, 2, 3), keepdims=True) + EPS
    )
    return (w * sigma_inv.astype(np.float32)).astype(np.float32)


def pack_weights(w_norm):
    """Pack normalized [O,I,kh,kw] weights into the [128, 9, 64] SBUF image.

    Column group g = kw for the even-row mains (rows 0:64 <- kh=1,
    rows 64:128 <- kh=2), g = 3+kw for odd-row mains (kh=0 / kh=1),
    g = 6+kw for the leftovers (rows 0:64 <- kh=2, rows 64:128 <- kh=0).
    Each [64, 64] slice is wT = w[:, :, kh, kw].T (contraction dim first).
    """
    wt = np.transpose(w_norm, (2, 3, 1, 0))  # [kh, kw, in, out]
    wpack = np.zeros((128, 9, 64), dtype=np.float32)
    for kw in range(3):
        wpack[0:64, kw] = wt[1, kw]
        wpack[64:128, kw] = wt[2, kw]
        wpack[0:64, 3 + kw] = wt[0, kw]
        wpack[64:128, 3 + kw] = wt[1, kw]
        wpack[0:64, 6 + kw] = wt[2, kw]
        wpack[64:128, 6 + kw] = wt[0, kw]
    return wpack


_NC_CACHE = {}


def _get_nc(bpc, h, block=64):
    key = (bpc, h, block)
    if key not in _NC_CACHE:
        _NC_CACHE[key] = build_nc(bpc, h, block)
    return _NC_CACHE[key]


def split_parity(x_f32):
    """[b, c, h, w] f32 -> bf16 [b, c, 2, h//2, w+2]: row parity split plus
    zero border columns (p=0 even rows, p=1 odd rows)."""
    import ml_dtypes

    b, c, h, w = x_f32.shape
    xb = x_f32.astype(ml_dtypes.bfloat16)
    xP = np.zeros((b, c, 2, h // 2, w + 2), dtype=ml_dtypes.bfloat16)
    xP[:, :, 0, :, 1:-1] = xb[:, :, 0::2]
    xP[:, :, 1, :, 1:-1] = xb[:, :, 1::2]
    return xP


def merge_parity(outP):
    """Device [b, p, hg, ud, c, uu, up, w] -> fp32 [b, c, h, w] with
    h = 2*(16*hg + 4*uu + 2*ud + up) + p."""
    b, p, hg, ud, c, uu, up, w = outP.shape
    o = np.transpose(outP, (0, 4, 2, 5, 3, 6, 1, 7))  # b c hg uu ud up p w
    return np.ascontiguousarray(
        o.reshape(b, c, hg * uu * ud * up * p, w), dtype=np.float32
    )


def kernel(x, weight):
    import ml_dtypes
    from concourse import bass_utils

    x = np.asarray(x, dtype=np.float32)
    weight = np.asarray(weight, dtype=np.float32)
    assert x.shape == (B_FULL, IN_F, H_FULL, W), x.shape

    xP = split_parity(x)
    wpack = pack_weights(normalize_weight(weight)).astype(ml_dtypes.bfloat16)
    bpc = B_FULL // N_CORES
    nc = _get_nc(bpc, H_FULL)
    in_maps = [
        {"x": xP[i * bpc : (i + 1) * bpc], "wpack": wpack} for i in range(N_CORES)
    ]
    res = bass_utils.run_bass_kernel_spmd(nc, in_maps, core_ids=list(range(N_CORES)))
    return np.concatenate([merge_parity(r["out"]) for r in res.results], axis=0)


# revision 30
# speedup vs baseline: 1.0525x; 1.0093x over previous
"""Trainium2 Bass kernel for Conv2dWeightModulate (no style).

The reference computes an equalized-lr + demodulated 3x3 conv:
    w = weight * C_EQ;  w *= rsqrt(sum(w^2, (I,K,K)) + eps);  out = conv2d(x, w, pad=1)

The tiny weight normalization runs on host (numpy); the conv runs on 8
NeuronCores, data-parallel over the batch (2 images per core).

Host-side data layout: x is cast to bf16 and split by row parity into
xP[b, c, p, h2, w] (= x[b, c, 2*h2+p, w]) so every DMA reads long
contiguous spans; the device writes a parity-split fp16 output that the
host re-interleaves (and upcasts to fp32).

Device kernel layout (per core):
  x is stored in SBUF parity-interleaved: partitions 0-63 hold the 64
  channels of even image rows, partitions 64-127 the odd rows, with each
  row padded to 258 columns (zero borders give the conv its padding).
  Chunk column s of a block with row base R holds:
      half A (parts 0:64):   x row R + 2(s-1)
      half B (parts 64:128): x row R + 2s - 1
  so chunk s aligns x rows (2j, 2j+1) vertically.  A 3x3 conv then becomes,
  per pair of same-parity output rows (one 512-wide matmul free dim):
      - even rows: K=128 matmul (taps kh=1+kh=2) x3 kw  +  K=64 (kh=0) x3
      - odd rows:  K=128 matmul (taps kh=0+kh=1) x3 kw  +  K=64 (kh=2) x3
  Adjacent row-pairs are col-tiled (tile_position via PSUM base partition
  64) so the pair runs concurrently on disjoint PE column groups; the
  K=64 leftovers of even/odd chunks land on disjoint PE quadrants and run
  4-way concurrent.  Each even/odd PSUM pair shares one full-width
  [128, 2, W] PSUM tile (halves on disjoint partition ranges), so copies
  are full-128-lane and only 2 PSUM banks are live per row-group,
  letting the pool double-buffer 4 deep.  Accumulation is fp32 in PSUM;
  outputs staged through SBUF as fp16 in 32-row groups and DMAed out.
"""

import numpy as np

IN_F = 64
OUT_F = 64
KS = 3
EPS = 1e-05
C_EQ = 1.0 / np.sqrt(IN_F * KS * KS)

B_FULL = 16
H_FULL = 256
W = 256
N_CORES = 8
CW = W + 2  # padded row width


def build_nc(bpc, h, block=64):
    """Build the per-core Bass program: bpc images of [64, h, 256] each."""
    from concourse import bacc
    import concourse.mybir as mybir
    from concourse.tile import TileContext

    assert h % block == 0 and block % 32 == 0
    nblk = h // block
    ngrp = block // 32  # 32-row output staging groups per block
    sch = block // 2 + 2  # chunk columns per x tile
    f32 = mybir.dt.float32
    bf16 = mybir.dt.bfloat16
    f16 = mybir.dt.float16

    nc = bacc.Bacc("TRN2", target_bir_lowering=False, debug=False)
    x = nc.dram_tensor("x", [bpc, IN_F, 2, h // 2, CW], bf16, kind="ExternalInput")
    wp = nc.dram_tensor("wpack", [128, 9, 64], bf16, kind="ExternalInput")
    # output stays in staging order: h = 2*(16*hg + 4*uu + 2*ud + up) + p,
    # so each 32-row group+parity is one fully contiguous 512KB DMA
    # (partition-major, 4KB contiguous per partition); host reassembles.
    # A staging tile's partition half selects ud, its inner row dim is up.
    out = nc.dram_tensor(
        "out", [bpc, 2, h // 32, 2, OUT_F, 4, 2, W], f16, kind="ExternalOutput"
    )
    outr = out.ap().rearrange("b p hg ud c uu up w -> b p hg (ud c) uu up w")

    with TileContext(nc) as tc:
        with (
            tc.tile_pool(name="xp", bufs=5) as xpool,
            tc.tile_pool(name="wpool", bufs=1) as wpool,
            tc.tile_pool(name="st", bufs=10) as spool,
            tc.tile_pool(name="ps", bufs=4, space="PSUM") as ppool,
        ):
            wt = wpool.tile([128, 9, 64], bf16)
            # weights lead the scalar HWDGE queue; the sync HWDGE queue
            # starts on the first x tile in parallel
            nc.scalar.dma_start(out=wt[:], in_=wp.ap())
            # ~3.6us of dummy matmuls on a zeroed tile warm the PE HAM clock
            # gate (cold = 1.2GHz) while the first x pieces are in flight, so
            # real matmuls start at the full 2.4GHz
            warm = spool.tile([128, 2, W], bf16, tag="warm", bufs=1)
            nc.gpsimd.memset(warm[:], 0.0)
            psW = ppool.tile([128, 2, W], f32, tag="psE")
            for _ in range(6):
                nc.tensor.matmul(psW[0:64], warm[:, 0, 0:64], warm[:], start=True, stop=True)
            for b in range(bpc):
                for blk in range(nblk):
                    R = blk * block
                    h0 = R // 2
                    xt = xpool.tile([128, sch, CW], bf16, tag="xt")
                    # host pre-pads rows to 258 with zero borders, so every
                    # transfer is one contiguous span per channel
                    # half A <- even x rows R .. R+block (chunks 1..sch-1)
                    # half B <- odd x rows R-1 .. R+block-1 (chunks 0..sch-2)
                    if blk == nblk - 1:
                        a_lo, a_hi = h0, h0 + sch - 2
                        nc.gpsimd.memset(xt[0:64, sch - 1, :], 0.0)
                    else:
                        a_lo, a_hi = h0, h0 + sch - 1
                    if blk == 0:
                        nc.gpsimd.memset(xt[64:128, 0, :], 0.0)
                        b_s, b_lo, b_hi = 1, 0, sch - 2
                    else:
                        b_s, b_lo, b_hi = 0, h0 - 1, h0 + sch - 2
                    if b == 0 and blk == 0:
                        # tiny leading pieces: the very first matmuls gate on
                        # ~130KB (2 rows/half), the rest ladders in behind
                        cuts = [2, 6, 14]
                    else:
                        cuts = [sch // 2]
                    a_cut = [a_lo] + [min(a_lo + c, a_hi) for c in cuts] + [a_hi]
                    b_cut = [b_lo] + [min(b_lo + c, b_hi) for c in cuts] + [b_hi]
                    for lo, hi in zip(a_cut, a_cut[1:]):
                        if hi > lo:
                            nc.sync.dma_start(
                                out=xt[0:64, 1 + (lo - a_lo) : 1 + (hi - a_lo), :],
                                in_=x.ap()[b, :, 0, lo:hi, :],
                            )
                    # first block's B half rides the scalar HWDGE queue (in
                    # parallel with A on sync) for a fast start; afterwards
                    # scalar belongs to E-output, so B stays on sync to keep
                    # input ordered ahead of everything else there
                    beng = nc.scalar if (b == 0 and blk == 0) else nc.sync
                    for lo, hi in zip(b_cut, b_cut[1:]):
                        if hi > lo:
                            beng.dma_start(
                                out=xt[64:128, b_s + (lo - b_lo) : b_s + (hi - b_lo), :],
                                in_=x.ap()[b, :, 1, lo:hi, :],
                            )
                    for g in range(ngrp):
                        hg = (h0 + 16 * g) // 16
                        stE = spool.tile([128, 4, 2, W], f16, tag="stE")
                        stO = spool.tile([128, 4, 2, W], f16, tag="stO")
                        for uu in range(4):
                            r0 = R + 32 * g + 8 * uu
                            s0 = (r0 - R) // 2 + 1  # A-chunk of x row r0
                            psE = ppool.tile([128, 2, W], f32, tag="psE")
                            psO = ppool.tile([128, 2, W], f32, tag="psO")
                            # E mains: out rows (r0, r0+2 | r0+4, r0+6), taps kh=1,2
                            # O mains: out rows (r0+1, r0+3 | r0+5, r0+7), taps kh=0,1
                            first = b == 0 and blk == 0 and g == 0 and uu == 0
                            if first:
                                # the very first unit runs all lower-half (E1/
                                # O1) matmuls first: they gate on the tiny
                                # 2-row lead piece, the upper half on piece 2
                                halves = [(0,), (1,)]
                            else:
                                halves = [(0, 1)]
                            for hs in halves:
                                for eo in range(2):
                                    pst = psE if eo == 0 else psO
                                    for kw in range(3):
                                        st_ = kw == 0
                                        for hf in hs:
                                            nc.tensor.matmul(
                                                pst[64 * hf : 64 * hf + 64],
                                                wt[:, 3 * eo + kw, :],
                                                xt[:, s0 + 2 * hf : s0 + 2 * hf + 2, kw : kw + W],
                                                start=st_, stop=False,
                                            )
                            # leftovers (4-way concurrent PE quadrants):
                            # E: tap kh=0 from half B; O: tap kh=2 from half A
                            for kw in range(3):
                                sp_ = kw == 2
                                nc.tensor.matmul(
                                    psE[0:64], wt[64:128, 6 + kw, :],
                                    xt[64:128, s0 - 1 : s0 + 1, kw : kw + W],
                                    start=False, stop=sp_,
                                )
                                nc.tensor.matmul(
                                    psE[64:128], wt[64:128, 6 + kw, :],
                                    xt[64:128, s0 + 1 : s0 + 3, kw : kw + W],
                                    start=False, stop=sp_,
                                )
                                nc.tensor.matmul(
                                    psO[0:64], wt[0:64, 6 + kw, :],
                                    xt[0:64, s0 + 1 : s0 + 3, kw : kw + W],
                                    start=False, stop=sp_,
                                )
                                nc.tensor.matmul(
                                    psO[64:128], wt[0:64, 6 + kw, :],
                                    xt[0:64, s0 + 3 : s0 + 5, kw : kw + W],
                                    start=False, stop=sp_,
                                )
                            nc.scalar.copy(stE[:, uu], psE[:])
                            nc.vector.tensor_copy(out=stO[:, uu], in_=psO[:])
                        # output DMAs ride the gpsimd SWDGE queue: its FIFO
                        # carries nothing matmul-critical, so completion
                        # guards never stall compute (the copy engines must
                        # not host these — their FIFOs would convoy).  Once
                        # input is done the sync HWDGE queue is free, so the
                        # final blocks' E groups drain there for a fast tail.
                        last_grp = b == bpc - 1 and blk == nblk - 1 and g == ngrp - 1
                        if b == bpc - 1 and blk >= nblk - 2:
                            nc.sync.dma_start(out=outr[b, 0, hg], in_=stE[:])
                        else:
                            nc.gpsimd.dma_start(out=outr[b, 0, hg], in_=stE[:])
                        if last_grp:
                            # final group: no copies remain behind this issue,
                            # so the fast scalar HWDGE queue is convoy-safe
                            nc.scalar.dma_start(out=outr[b, 1, hg], in_=stO[:])
                        else:
                            nc.gpsimd.dma_start(out=outr[b, 1, hg], in_=stO[:])
    nc.compile()
    return nc


def normalize_weight(weight):
    """Host-side equalized-lr + demodulation of the [O,I,3,3] weight."""
    w = np.asarray(weight, dtype=np.float32) * np.float32(C_EQ)
    sigma_inv = 1.0 / np.sqrt(
        np.sum((w * w).astype(np.float32), axis=(1, 2, 3), keepdims=True) + EPS
    )
    return (w * sigma_inv.astype(np.float32)).astype(np.float32)


def pack_weights(w_norm):
    """Pack normalized [O,I,kh,kw] weights into the [128, 9, 64] SBUF image.

    Column group g = kw for the even-row mains (rows 0:64 <- kh=1,
    rows 64:128 <- kh=2), g = 3+kw for odd-row mains (kh=0 / kh=1),
    g = 6+kw for the leftovers (rows 0:64 <- kh=2, rows 64:128 <- kh=0).
    Each [64, 64] slice is wT = w[:, :, kh, kw].T (contraction dim first).
    """
    wt = np.transpose(w_norm, (2, 3, 1, 0))  # [kh, kw, in, out]
    wpack = np.zeros((128, 9, 64), dtype=np.float32)
    for kw in range(3):
        wpack[0:64, kw] = wt[1, kw]
        wpack[64:128, kw] = wt[2, kw]
        wpack[0:64, 3 + kw] = wt[0, kw]
        wpack[64:128, 3 + kw] = wt[1, kw]
        wpack[0:64, 6 + kw] = wt[2, kw]
        wpack[64:128, 6 + kw] = wt[0, kw]
    return wpack


_NC_CACHE = {}


def _get_nc(bpc, h, block=64):
    key = (bpc, h, block)
    if key not in _NC_CACHE:
        _NC_CACHE[key] = build_nc(bpc, h, block)
    return _NC_CACHE[key]


def split_parity(x_f32):
    """[b, c, h, w] f32 -> bf16 [b, c, 2, h//2, w+2]: row parity split plus
    zero border columns (p=0 even rows, p=1 odd rows)."""
    import ml_dtypes

    b, c, h, w = x_f32.shape
    xb = x_f32.astype(ml_dtypes.bfloat16)
    xP = np.zeros((b, c, 2, h // 2, w + 2), dtype=ml_dtypes.bfloat16)
    xP[:, :, 0, :, 1:-1] = xb[:, :, 0::2]
    xP[:, :, 1, :, 1:-1] = xb[:, :, 1::2]
    return xP


def merge_parity(outP):
    """Device [b, p, hg, ud, c, uu, up, w] -> fp32 [b, c, h, w] with
    h = 2*(16*hg + 4*uu + 2*ud + up) + p."""
    b, p, hg, ud, c, uu, up, w = outP.shape
    o = np.transpose(outP, (0, 4, 2, 5, 3, 6, 1, 7))  # b c hg uu ud up p w
    return np.ascontiguousarray(
        o.reshape(b, c, hg * uu * ud * up * p, w), dtype=np.float32
    )


def kernel(x, weight):
    import ml_dtypes
    from concourse import bass_utils

    x = np.asarray(x, dtype=np.float32)
    weight = np.asarray(weight, dtype=np.float32)
    assert x.shape == (B_FULL, IN_F, H_FULL, W), x.shape

    xP = split_parity(x)
    wpack = pack_weights(normalize_weight(weight)).astype(ml_dtypes.bfloat16)
    bpc = B_FULL // N_CORES
    nc = _get_nc(bpc, H_FULL)
    in_maps = [
        {"x": xP[i * bpc : (i + 1) * bpc], "wpack": wpack} for i in range(N_CORES)
    ]
    res = bass_utils.run_bass_kernel_spmd(nc, in_maps, core_ids=list(range(N_CORES)))
    return np.concatenate([merge_parity(r["out"]) for r in res.results], axis=0)


# revision 32
# speedup vs baseline: 1.0658x; 1.0127x over previous
"""Trainium2 Bass kernel for Conv2dWeightModulate (no style).

The reference computes an equalized-lr + demodulated 3x3 conv:
    w = weight * C_EQ;  w *= rsqrt(sum(w^2, (I,K,K)) + eps);  out = conv2d(x, w, pad=1)

The tiny weight normalization runs on host (numpy); the conv runs on 8
NeuronCores, data-parallel over the batch (2 images per core).

Host-side data layout: x is cast to bf16 and split by row parity into
xP[b, c, p, h2, w] (= x[b, c, 2*h2+p, w]) so every DMA reads long
contiguous spans; the device writes a parity-split fp16 output that the
host re-interleaves (and upcasts to fp32).

Device kernel layout (per core):
  x is stored in SBUF parity-interleaved: partitions 0-63 hold the 64
  channels of even image rows, partitions 64-127 the odd rows, with each
  row padded to 258 columns (zero borders give the conv its padding).
  Chunk column s of a block with row base R holds:
      half A (parts 0:64):   x row R + 2(s-1)
      half B (parts 64:128): x row R + 2s - 1
  so chunk s aligns x rows (2j, 2j+1) vertically.  A 3x3 conv then becomes,
  per pair of same-parity output rows (one 512-wide matmul free dim):
      - even rows: K=128 matmul (taps kh=1+kh=2) x3 kw  +  K=64 (kh=0) x3
      - odd rows:  K=128 matmul (taps kh=0+kh=1) x3 kw  +  K=64 (kh=2) x3
  Adjacent row-pairs are col-tiled (tile_position via PSUM base partition
  64) so the pair runs concurrently on disjoint PE column groups; the
  K=64 leftovers of even/odd chunks land on disjoint PE quadrants and run
  4-way concurrent.  Each even/odd PSUM pair shares one full-width
  [128, 2, W] PSUM tile (halves on disjoint partition ranges), so copies
  are full-128-lane and only 2 PSUM banks are live per row-group,
  letting the pool double-buffer 4 deep.  Accumulation is fp32 in PSUM;
  outputs staged through SBUF as fp16 in 32-row groups and DMAed out.
"""

import numpy as np

IN_F = 64
OUT_F = 64
KS = 3
EPS = 1e-05
C_EQ = 1.0 / np.sqrt(IN_F * KS * KS)

B_FULL = 16
H_FULL = 256
W = 256
N_CORES = 8
CW = W + 2  # padded row width


def build_nc(bpc, h, block=64):
    """Build the per-core Bass program: bpc images of [64, h, 256] each."""
    from concourse import bacc
    import concourse.mybir as mybir
    from concourse.tile import TileContext

    assert h % block == 0 and block % 32 == 0
    nblk = h // block
    ngrp = block // 32  # 32-row output staging groups per block
    sch = block // 2 + 2  # chunk columns per x tile
    f32 = mybir.dt.float32
    bf16 = mybir.dt.bfloat16
    f16 = mybir.dt.float16

    nc = bacc.Bacc("TRN2", target_bir_lowering=False, debug=False)
    x = nc.dram_tensor("x", [bpc, IN_F, 2, h // 2, CW], bf16, kind="ExternalInput")
    wp = nc.dram_tensor("wpack", [128, 9, 64], bf16, kind="ExternalInput")
    # output stays in staging order: h = 2*(16*hg + 4*uu + 2*ud + up) + p,
    # so each 32-row group+parity is one fully contiguous 512KB DMA
    # (partition-major, 4KB contiguous per partition); host reassembles.
    # A staging tile's partition half selects ud, its inner row dim is up.
    out = nc.dram_tensor(
        "out", [bpc, 2, h // 32, 2, OUT_F, 4, 2, W], f16, kind="ExternalOutput"
    )
    outr = out.ap().rearrange("b p hg ud c uu up w -> b p hg (ud c) uu up w")

    with TileContext(nc) as tc:
        with (
            tc.tile_pool(name="xp", bufs=5) as xpool,
            tc.tile_pool(name="wpool", bufs=1) as wpool,
            tc.tile_pool(name="st", bufs=10) as spool,
            tc.tile_pool(name="ps", bufs=4, space="PSUM") as ppool,
        ):
            wt = wpool.tile([128, 9, 64], bf16)
            # weights lead the scalar HWDGE queue; the sync HWDGE queue
            # starts on the first x tile in parallel
            nc.scalar.dma_start(out=wt[:], in_=wp.ap())
            # ~3.6us of dummy matmuls on a zeroed tile warm the PE HAM clock
            # gate (cold = 1.2GHz) while the first x pieces are in flight, so
            # real matmuls start at the full 2.4GHz
            warm = spool.tile([128, 2, W], bf16, tag="warm", bufs=1)
            nc.gpsimd.memset(warm[:], 0.0)
            psW = ppool.tile([128, 2, W], f32, tag="psE")
            for _ in range(10):
                nc.tensor.matmul(psW[0:64], warm[:, 0, 0:64], warm[:], start=True, stop=True)
            for b in range(bpc):
                for blk in range(nblk):
                    R = blk * block
                    h0 = R // 2
                    xt = xpool.tile([128, sch, CW], bf16, tag="xt")
                    # host pre-pads rows to 258 with zero borders, so every
                    # transfer is one contiguous span per channel
                    # half A <- even x rows R .. R+block (chunks 1..sch-1)
                    # half B <- odd x rows R-1 .. R+block-1 (chunks 0..sch-2)
                    if blk == nblk - 1:
                        a_lo, a_hi = h0, h0 + sch - 2
                        nc.gpsimd.memset(xt[0:64, sch - 1, :], 0.0)
                    else:
                        a_lo, a_hi = h0, h0 + sch - 1
                    if blk == 0:
                        nc.gpsimd.memset(xt[64:128, 0, :], 0.0)
                        b_s, b_lo, b_hi = 1, 0, sch - 2
                    else:
                        b_s, b_lo, b_hi = 0, h0 - 1, h0 + sch - 2
                    if b == 0 and blk == 0:
                        # tiny leading pieces: the very first matmuls gate on
                        # ~130KB (2 rows/half), the rest ladders in behind
                        cuts = [2, 6, 14, 22]
                    else:
                        cuts = [sch // 2]
                    a_cut = [a_lo] + [min(a_lo + c, a_hi) for c in cuts] + [a_hi]
                    b_cut = [b_lo] + [min(b_lo + c, b_hi) for c in cuts] + [b_hi]
                    for lo, hi in zip(a_cut, a_cut[1:]):
                        if hi > lo:
                            nc.sync.dma_start(
                                out=xt[0:64, 1 + (lo - a_lo) : 1 + (hi - a_lo), :],
                                in_=x.ap()[b, :, 0, lo:hi, :],
                            )
                    # first block's B half rides the scalar HWDGE queue (in
                    # parallel with A on sync) for a fast start; afterwards
                    # scalar belongs to E-output, so B stays on sync to keep
                    # input ordered ahead of everything else there
                    beng = nc.scalar if (b == 0 and blk == 0) else nc.sync
                    for lo, hi in zip(b_cut, b_cut[1:]):
                        if hi > lo:
                            beng.dma_start(
                                out=xt[64:128, b_s + (lo - b_lo) : b_s + (hi - b_lo), :],
                                in_=x.ap()[b, :, 1, lo:hi, :],
                            )
                    for g in range(ngrp):
                        hg = (h0 + 16 * g) // 16
                        stE = spool.tile([128, 4, 2, W], f16, tag="stE")
                        stO = spool.tile([128, 4, 2, W], f16, tag="stO")
                        for uu in range(4):
                            r0 = R + 32 * g + 8 * uu
                            s0 = (r0 - R) // 2 + 1  # A-chunk of x row r0
                            psE = ppool.tile([128, 2, W], f32, tag="psE")
                            psO = ppool.tile([128, 2, W], f32, tag="psO")
                            # E mains: out rows (r0, r0+2 | r0+4, r0+6), taps kh=1,2
                            # O mains: out rows (r0+1, r0+3 | r0+5, r0+7), taps kh=0,1
                            first = b == 0 and blk == 0 and g == 0 and uu == 0
                            if first:
                                # the very first unit runs all lower-half (E1/
                                # O1) matmuls first: they gate on the tiny
                                # 2-row lead piece, the upper half on piece 2
                                halves = [(0,), (1,)]
                            else:
                                halves = [(0, 1)]
                            for hs in halves:
                                for eo in range(2):
                                    pst = psE if eo == 0 else psO
                                    for kw in range(3):
                                        st_ = kw == 0
                                        for hf in hs:
                                            nc.tensor.matmul(
                                                pst[64 * hf : 64 * hf + 64],
                                                wt[:, 3 * eo + kw, :],
                                                xt[:, s0 + 2 * hf : s0 + 2 * hf + 2, kw : kw + W],
                                                start=st_, stop=False,
                                            )
                            # leftovers (4-way concurrent PE quadrants):
                            # E: tap kh=0 from half B; O: tap kh=2 from half A
                            for kw in range(3):
                                sp_ = kw == 2
                                nc.tensor.matmul(
                                    psE[0:64], wt[64:128, 6 + kw, :],
                                    xt[64:128, s0 - 1 : s0 + 1, kw : kw + W],
                                    start=False, stop=sp_,
                                )
                                nc.tensor.matmul(
                                    psE[64:128], wt[64:128, 6 + kw, :],
                                    xt[64:128, s0 + 1 : s0 + 3, kw : kw + W],
                                    start=False, stop=sp_,
                                )
                                nc.tensor.matmul(
                                    psO[0:64], wt[0:64, 6 + kw, :],
                                    xt[0:64, s0 + 1 : s0 + 3, kw : kw + W],
                                    start=False, stop=sp_,
                                )
                                nc.tensor.matmul(
                                    psO[64:128], wt[0:64, 6 + kw, :],
                                    xt[0:64, s0 + 3 : s0 + 5, kw : kw + W],
                                    start=False, stop=sp_,
                                )
                            nc.scalar.copy(stE[:, uu], psE[:])
                            nc.vector.tensor_copy(out=stO[:, uu], in_=psO[:])
                        # output DMAs ride the gpsimd SWDGE queue: its FIFO
                        # carries nothing matmul-critical, so completion
                        # guards never stall compute (the copy engines must
                        # not host these — their FIFOs would convoy).  Once
                        # input is done the sync HWDGE queue is free, so the
                        # final blocks' E groups drain there for a fast tail.
                        last_grp = b == bpc - 1 and blk == nblk - 1 and g == ngrp - 1
                        if b == bpc - 1 and blk >= nblk - 2:
                            nc.sync.dma_start(out=outr[b, 0, hg], in_=stE[:])
                        else:
                            nc.gpsimd.dma_start(out=outr[b, 0, hg], in_=stE[:])
                        if last_grp:
                            # final group: no copies remain behind this issue,
                            # so the fast scalar HWDGE queue is convoy-safe
                            nc.scalar.dma_start(out=outr[b, 1, hg], in_=stO[:])
                        else:
                            nc.gpsimd.dma_start(out=outr[b, 1, hg], in_=stO[:])
    nc.compile()
    return nc


def normalize_weight(weight):
    """Host-side equalized-lr + demodulation of the [O,I,3,3] weight."""
    w = np.asarray(weight, dtype=np.float32) * np.float32(C_EQ)
    sigma_inv = 1.0 / np.sqrt(
        np.sum((w * w).astype(np.float32), axis=(1, 2, 3), keepdims=True) + EPS
    )
    return (w * sigma_inv.astype(np.float32)).astype(np.float32)


def pack_weights(w_norm):
    """Pack normalized [O,I,kh,kw] weights into the [128, 9, 64] SBUF image.

    Column group g = kw for the even-row mains (rows 0:64 <- kh=1,
    rows 64:128 <- kh=2), g = 3+kw for odd-row mains (kh=0 / kh=1),
    g = 6+kw for the leftovers (rows 0:64 <- kh=2, rows 64:128 <- kh=0).
    Each [64, 64] slice is wT = w[:, :, kh, kw].T (contraction dim first).
    """
    wt = np.transpose(w_norm, (2, 3, 1, 0))  # [kh, kw, in, out]
    wpack = np.zeros((128, 9, 64), dtype=np.float32)
    for kw in range(3):
        wpack[0:64, kw] = wt[1, kw]
        wpack[64:128, kw] = wt[2, kw]
        wpack[0:64, 3 + kw] = wt[0, kw]
        wpack[64:128, 3 + kw] = wt[1, kw]
        wpack[0:64, 6 + kw] = wt[2, kw]
        wpack[64:128, 6 + kw] = wt[0, kw]
    return wpack


_NC_CACHE = {}


def _get_nc(bpc, h, block=64):
    key = (bpc, h, block)
    if key not in _NC_CACHE:
        _NC_CACHE[key] = build_nc(bpc, h, block)
    return _NC_CACHE[key]


def split_parity(x_f32):
    """[b, c, h, w] f32 -> bf16 [b, c, 2, h//2, w+2]: row parity split plus
    zero border columns (p=0 even rows, p=1 odd rows)."""
    import ml_dtypes

    b, c, h, w = x_f32.shape
    xb = x_f32.astype(ml_dtypes.bfloat16)
    xP = np.zeros((b, c, 2, h // 2, w + 2), dtype=ml_dtypes.bfloat16)
    xP[:, :, 0, :, 1:-1] = xb[:, :, 0::2]
    xP[:, :, 1, :, 1:-1] = xb[:, :, 1::2]
    return xP


def merge_parity(outP):
    """Device [b, p, hg, ud, c, uu, up, w] -> fp32 [b, c, h, w] with
    h = 2*(16*hg + 4*uu + 2*ud + up) + p."""
    b, p, hg, ud, c, uu, up, w = outP.shape
    o = np.transpose(outP, (0, 4, 2, 5, 3, 6, 1, 7))  # b c hg uu ud up p w
    return np.ascontiguousarray(
        o.reshape(b, c, hg * uu * ud * up * p, w), dtype=np.float32
    )


def kernel(x, weight):
    import ml_dtypes
    from concourse import bass_utils

    x = np.asarray(x, dtype=np.float32)
    weight = np.asarray(weight, dtype=np.float32)
    assert x.shape == (B_FULL, IN_F, H_FULL, W), x.shape

    xP = split_parity(x)
    wpack = pack_weights(normalize_weight(weight)).astype(ml_dtypes.bfloat16)
    bpc = B_FULL // N_CORES
    nc = _get_nc(bpc, H_FULL)
    in_maps = [
        {"x": xP[i * bpc : (i + 1) * bpc], "wpack": wpack} for i in range(N_CORES)
    ]
    res = bass_utils.run_bass_kernel_spmd(nc, in_maps, core_ids=list(range(N_CORES)))
    return np.concatenate([merge_parity(r["out"]) for r in res.results], axis=0)
